# revision 1
# baseline (speedup 1.0000x reference)
"""Masked max-pool (mention representation) Trainium2 kernel.

out[b, m, :] = max_s( h[b, s, :] + (mask[b, m, s] ? 0 : -1e30) )   [B,M,H]

Shapes (hardcoded): h [2, 1024, 768] f32, mention_masks [2, 128, 1024] i32,
out [2, 128, 768] f32.

Sharding: 8 cores, core = (b, m-chunk): b = core // 4, 32 mentions per core.
Each core sees hT [768, 1024] (host-pretransposed) and neg [32, 1024]
(host-precomputed additive mask values in f32).

Per-core program:
  - DMA hT into 6 SBUF tiles [128ch, 1024s], neg into SBUF [32, 1024].
  - Per mention m: PE K=1 matmul (ones[1,128].T @ neg[m:m+1, :]) broadcasts
    neg[m, :] across 128 partitions into PSUM [128, 1024] (two N=512 matmuls).
  - Per (m, g in 6): DVE tensor_tensor_reduce computes
      scratch = hT_g + neg_rep ; out_col = max_free(scratch)
    in one fused 1x pass, writing out[g][:, m] = the masked max for 128
    channels. Exact fp32, bit-identical to the reference reduction.
  - DMA out tiles [128, 32] to DRAM outT [768, 32] (contiguous); host
    transposes back.
"""

import ml_dtypes
import numpy as np

B, S, H = 2, 1024, 768
M = 128
N_CORES = 8
M_PER_CORE = M // (N_CORES // B)  # 32
G = H // 128  # 6 channel groups

_NC = None
_LAST_RESULTS = None


def _build_nc(repeat=1):
    import concourse.bacc as bacc
    import concourse.mybir as mybir
    import concourse.tile as tile

    f32 = mybir.dt.float32

    bf16 = mybir.dt.bfloat16
    nc = bacc.Bacc(
        "TRN2",
        target_bir_lowering=False,
        debug=False,
        enable_asserts=False,
        num_devices=N_CORES,
    )
    hT = nc.dram_tensor("ht", [H, S], f32, kind="ExternalInput")
    neg = nc.dram_tensor("neg", [1, M_PER_CORE * S], bf16, kind="ExternalInput")
    outT = nc.dram_tensor("outt", [H, M_PER_CORE], f32, kind="ExternalOutput")

    with tile.TileContext(nc) as tc:
        with (
            tc.tile_pool(name="hpool", bufs=1) as hpool,
            tc.tile_pool(name="misc", bufs=1) as misc,
            tc.tile_pool(name="scratch", bufs=2) as spool,
            tc.tile_pool(name="psum", bufs=2, space="PSUM") as ppool,
        ):
            h_tiles = []
            for g in range(G):
                t = hpool.tile([128, S], f32, tag=f"h{g}", name=f"h{g}")
                nc.sync.dma_start(t[:], hT.ap()[g * 128 : (g + 1) * 128, :])
                h_tiles.append(t)

            negt = misc.tile([1, M_PER_CORE * S], bf16, tag="neg")
            nc.sync.dma_start(negt[:], neg.ap()[:, :])

            ones = misc.tile([1, 128], bf16, tag="ones")
            nc.gpsimd.memset(ones[:], 1.0)

            out_tiles = []
            for g in range(G):
                out_tiles.append(
                    misc.tile([128, M_PER_CORE], f32, tag=f"o{g}", name=f"o{g}")
                )

            for rep in range(repeat):
              for m in range(M_PER_CORE):
                nrep = ppool.tile([128, S], f32, tag="nrep")
                for half in range(2):
                    lo = half * 512
                    nc.tensor.matmul(
                        nrep[:, lo : lo + 512],
                        ones[:],
                        negt[0:1, m * S + lo : m * S + lo + 512],
                        start=True,
                        stop=True,
                    )
                for g in range(G):
                    sc = spool.tile([128, S], f32, tag="sc")
                    nc.vector.tensor_tensor(
                        out=sc[:],
                        in0=h_tiles[g][:],
                        in1=nrep[:],
                        op=mybir.AluOpType.add,
                    )
                    nc.vector.tensor_reduce(
                        out=out_tiles[g][:, m : m + 1],
                        in_=sc[:],
                        axis=mybir.AxisListType.X,
                        op=mybir.AluOpType.max,
                    )

              for g in range(G):
                nc.sync.dma_start(
                    outT.ap()[g * 128 : (g + 1) * 128, :], out_tiles[g][:]
                )

    nc.compile()
    return nc


def _get_nc():
    global _NC
    if _NC is None:
        _NC = _build_nc()
    return _NC


def _make_in_maps(h, mention_masks):
    h = np.ascontiguousarray(np.asarray(h), dtype=np.float32)
    masks = np.asarray(mention_masks)
    neg = np.where(masks == 0, np.float32(-1e30), np.float32(0.0)).astype(np.float32)
    hT = np.ascontiguousarray(h.transpose(0, 2, 1))  # [B, H, S]
    in_maps = []
    for core in range(N_CORES):
        b, mc = divmod(core, N_CORES // B)
        in_maps.append(
            {
                "ht": hT[b],
                "neg": np.ascontiguousarray(
                    neg[b, mc * M_PER_CORE : (mc + 1) * M_PER_CORE]
                )
                .reshape(1, -1)
                .astype(ml_dtypes.bfloat16),
            }
        )
    return in_maps


def kernel(h, mention_masks, trace=False):
    global _LAST_RESULTS
    from concourse.bass_utils import run_bass_kernel_spmd

    nc = _get_nc()
    in_maps = _make_in_maps(h, mention_masks)
    res = run_bass_kernel_spmd(
        nc, in_maps, core_ids=list(range(N_CORES)), trace=trace
    )
    _LAST_RESULTS = res
    out = np.empty((B, M, H), dtype=np.float32)
    for core in range(N_CORES):
        b, mc = divmod(core, N_CORES // B)
        out[b, mc * M_PER_CORE : (mc + 1) * M_PER_CORE] = res.results[core]["outt"].T
    return out



# revision 5
# speedup vs baseline: 25.3218x; 25.3218x over previous
"""Masked max-pool (mention representation) Trainium2 kernel.

out[b, m, :] = max_s( h[b, s, :] + (mask[b, m, s] ? 0 : -1e30) )   [B,M,H]

Shapes (hardcoded): h [2, 1024, 768] f32, mention_masks [2, 128, 1024] i32,
out [2, 128, 768] f32.

Sharding: 8 cores, core = (b, m-chunk): b = core // 4, 32 mentions per core.
Each core sees hT [768, 1024] bf16 (host-pretransposed + cast) and
neg [32, 1024] bf16 (host-precomputed additive mask values).

Per-core program (custom paged-scan DVE op, 2x bf16):
  - DMA hT into 6 SBUF tiles [128ch, 1, 1024s] bf16, neg into SBUF.
  - Mentions processed in chunks of CHUNK=4:
      per mention: PE K=1 matmul broadcasts neg[m, :] to PSUM [128,1024] f32;
      Act casts PSUM -> SBUF bf16 into the chunk's nrep strip.
      per channel-group g: ONE custom DVE instruction "PMAX_SCAN_ANT"
        in0 = h_g [128, CHUNK(pages, stride 0), 1024] (re-reads h per page)
        in1 = nrep strip [128, CHUNK*1024]
        out[k] = running max of (in0+in1), accumulator RESET at each page
        boundary (SUB_DIM_DONE uop) -> out[:, m, 1023] = masked max for
        mention m.  Hand-authored 2x_1p uop variant processes 2 bf16
        elements/lane/cycle (vs 1 for all stock 2-tensor reduce paths).
      Act extracts page-final columns -> out_tiles[g][:, chunk] f32.
  - DMA out tiles [128, 32] f32 to DRAM outT [768, 32]; host transposes back.

repeat>1 wraps the chunk loop in tc.For_i (for amortized-diff timing).
"""

import ml_dtypes
import numpy as np

B, S, H = 2, 1024, 768
M = 128
N_CORES = 8
M_PER_CORE = M // (N_CORES // B)  # 32
G = H // 128  # 6 channel groups
CHUNK = 4  # mentions per DVE instruction (pages)
N_CHUNKS = M_PER_CORE // CHUNK

USE_2X = True

_NC = None
_LAST_RESULTS = None
_OP = None


# --------------------------------------------------------------------------
# Custom DVE op: paged prefix-max scan of (Src0 + Src1).
#
#   out[p, s, k] = max(MaxNeg, max_{j<=k} (in0[p, s, j] + in1[p, s*N + j]))
#
# i.e. a running max along the innermost dim that RESETS at each page (s)
# boundary.  The page-final element out[p, s, N-1] is the per-page masked
# max.  No accum_out / READ_ACCUMULATOR needed - the result rides the
# ordinary output stream.
#
# 1x program (3 uops):          2x_1p program (3 uops, 2 elems/cycle):
#   blk0: ADD(src0, src1)         blk0: ADD(src0_lo, src1_lo)
#   blk1: MAX(acc*, blk0)         blk1: ADD(src0_hi, src1_hi); d0 <- lo_sum
#   blk2-7: bypass                blk2: MAX(hi_sum, d0=lo_sum)  (pair max)
#                                 blk3: MAX(acc*, pair)
#                                 blk4-7: bypass
#   acc* = CURR_ALU_OUT (steady uop) or MAX_NEG (entry/reseed uops).
#   uop0 = entry-reseed (1 elem) -> uop1 steady; SUB_DIM_DONE -> uop2
#   reseed (1 elem) -> uop1.  SRC_TENSOR_DONE -> idle.
# --------------------------------------------------------------------------


def _register_op():
    global _OP
    if _OP is not None:
        return _OP
    import concourse.dve_ops as dve_ops
    from concourse.dve_ops import DveOp
    from concourse.dve_spec import Spec, Src0, Src1, MaxNeg, scan
    from concourse.dve_spec import AluOp as SpecAluOp
    from concourse.dve_uop import (
        AluInp,
        AluOp,
        DveOpSpec,
        DelayInp,
        InpSel,
        OutPath,
        OutSel,
        Trigger,
        UopConfig,
        UopDpConfig,
    )

    NAME = "PMAX_SCAN_ANT"

    def _mk_1x(reseed):
        u = UopConfig()
        u.enable_input(InpSel.SRC_0, 1)  # -> PREV_DELAY_0
        u.enable_input(InpSel.SRC_1, 2)  # -> PREV_DELAY_1
        u.enable_input(InpSel.MAX_NEG, 3)  # -> PREV_DELAY_2
        u.require_inp0 = 1
        u.require_inp1 = 1
        u.enable_output(OutSel.ALU_OUT, OutPath.WR0_LO)
        u.datapath_config[0] = (
            UopDpConfig()
            .enable_alu(AluOp.ADD, AluInp.PREV_DELAY_0, AluInp.PREV_DELAY_1)
            .pass_through_delay(2)
        )
        acc_src = AluInp.PREV_DELAY_2 if reseed else AluInp.CURR_ALU_OUT
        u.datapath_config[1] = UopDpConfig().enable_alu(
            AluOp.MAX, acc_src, AluInp.PREV_ALU_OUT
        )
        for k in range(2, 8):
            u.datapath_config[k] = UopDpConfig().enable_alu(
                AluOp.BYPASS, AluInp.PREV_ALU_OUT, AluInp.PREV_ALU_OUT
            )
        return u

    def _mk_2x(reseed):
        u = UopConfig()
        u.enable_input(InpSel.SRC_0, 0)  # -> block0 ALU input (PREV_ALU_OUT)
        u.enable_input(InpSel.SRC_1, 1)  # -> PREV_DELAY_0
        u.enable_input(InpSel.SRC_0_HI, 2)  # -> PREV_DELAY_1
        u.enable_input(InpSel.SRC_1_HI, 3)  # -> PREV_DELAY_2
        u.enable_input(InpSel.MAX_NEG, 4)  # -> PREV_DELAY_3
        u.require_inp0 = 1
        u.require_inp1 = 1
        u.enable_output(OutSel.ALU_OUT, OutPath.WR0_LO)
        u.enable_output(OutSel.ALU_OUT, OutPath.WR0_HI)
        u.datapath_config[0] = (
            UopDpConfig()
            .enable_alu(AluOp.ADD, AluInp.PREV_ALU_OUT, AluInp.PREV_DELAY_0)
            .pass_through_delay(1, 2, 3)
        )
        u.datapath_config[1] = (
            UopDpConfig()
            .enable_alu(AluOp.ADD, AluInp.PREV_DELAY_1, AluInp.PREV_DELAY_2)
            .enable_delay_from_src(DelayInp.PREV_ALU_OUT, 0)
            .pass_through_delay(3)
        )
        u.datapath_config[2] = (
            UopDpConfig()
            .enable_alu(AluOp.MAX, AluInp.PREV_ALU_OUT, AluInp.PREV_DELAY_0)
            .pass_through_delay(3)
        )
        acc_src = AluInp.PREV_DELAY_3 if reseed else AluInp.CURR_ALU_OUT
        u.datapath_config[3] = UopDpConfig().enable_alu(
            AluOp.MAX, acc_src, AluInp.PREV_ALU_OUT
        )
        for k in range(4, 8):
            u.datapath_config[k] = UopDpConfig().enable_alu(
                AluOp.BYPASS, AluInp.PREV_ALU_OUT, AluInp.PREV_ALU_OUT
            )
        return u

    def _finalize(mk):
        u0 = mk(True)
        u0.repeat_count = 1
        u0.trigger = (Trigger.COUNT, Trigger.NONE, Trigger.NONE)
        u0.next_uop = (1, 0, 0)
        u1 = mk(False)
        u1.trigger = (Trigger.SRC_TENSOR_DONE, Trigger.SUB_DIM_DONE, Trigger.NONE)
        u1.next_uop = (0, 2, 0)
        u2 = mk(True)
        u2.repeat_count = 1
        u2.trigger = (Trigger.COUNT, Trigger.NONE, Trigger.NONE)
        u2.next_uop = (1, 0, 0)
        return [u0, u1, u2]

    row = max(dve_ops._SUB_OPCODE_FOR_NAME.values()) + 1
    assert row < 0x20
    op_spec = DveOpSpec(
        name=NAME,
        opcode=row,
        uops=_finalize(_mk_1x),
        uops_2x=_finalize(_mk_2x) if USE_2X else None,
        perf_max=1 if USE_2X else 0,
        rd1_en=True,
    )
    sha = op_spec.sha("v3")

    def _ref(in0, in1, s0, s1, imm2):
        x = np.ascontiguousarray(in0).astype(np.float32)
        y = np.ascontiguousarray(in1).astype(np.float32).reshape(x.shape)
        return np.maximum.accumulate(x + y, axis=-1)

    op = DveOp(
        NAME,
        Spec(body=scan(SpecAluOp.MAX, Src0 + Src1, init=MaxNeg), reference=_ref),
        subdim=True,
        uops_sha={"v3": sha},
    )
    dve_ops.OPS.append(op)
    dve_ops._SUB_OPCODE_FOR_NAME[NAME] = row
    dve_ops.CUSTOM_DVE_SPECS[NAME] = op.spec
    dve_ops._COMPILE_CACHE[(NAME, "v3")] = op_spec
    _OP = op
    return op


def _build_nc(repeat=1):
    import concourse.bacc as bacc
    import concourse.mybir as mybir
    import concourse.tile as tile

    op = _register_op()

    f32 = mybir.dt.float32
    bf16 = mybir.dt.bfloat16

    nc = bacc.Bacc(
        "TRN2",
        target_bir_lowering=False,
        debug=False,
        enable_asserts=False,
        num_devices=N_CORES,
    )
    hT = nc.dram_tensor("ht", [H, S], bf16, kind="ExternalInput")
    neg = nc.dram_tensor("neg", [1, M_PER_CORE * S], bf16, kind="ExternalInput")
    outT = nc.dram_tensor("outt", [H, M_PER_CORE], f32, kind="ExternalOutput")

    with tile.TileContext(nc) as tc:
        with (
            tc.tile_pool(name="hpool", bufs=1) as hpool,
            tc.tile_pool(name="misc", bufs=1) as misc,
            tc.tile_pool(name="nrpool", bufs=2) as nrpool,
            tc.tile_pool(name="scratch", bufs=2) as spool,
            tc.tile_pool(name="psum", bufs=2, space="PSUM") as ppool,
        ):
            h_tiles = []
            for g in range(G):
                t = hpool.tile([128, 1, S], bf16, tag=f"h{g}", name=f"h{g}")
                nc.sync.dma_start(t[:, 0:1, :], hT.ap()[g * 128 : (g + 1) * 128, :])
                h_tiles.append(t)

            negt = misc.tile([1, M_PER_CORE * S], bf16, tag="neg", name="negt")
            nc.sync.dma_start(negt[:], neg.ap()[:, :])

            ones = misc.tile([1, 128], bf16, tag="ones", name="ones")
            nc.gpsimd.memset(ones[:], 1.0)

            out_tiles = []
            for g in range(G):
                out_tiles.append(
                    misc.tile([128, M_PER_CORE], f32, tag=f"o{g}", name=f"o{g}")
                )

            def body():
                for c in range(N_CHUNKS):
                    nrep = nrpool.tile(
                        [128, CHUNK * S], bf16, tag="nrep", name="nrep"
                    )
                    for k in range(CHUNK):
                        m = c * CHUNK + k
                        np_t = ppool.tile([128, S], f32, tag="np", name="np_t")
                        for half in range(2):
                            lo = half * 512
                            nc.tensor.matmul(
                                np_t[:, lo : lo + 512],
                                ones[:],
                                negt[0:1, m * S + lo : m * S + lo + 512],
                                start=True,
                                stop=True,
                            )
                        nc.scalar.copy(nrep[:, k * S : (k + 1) * S], np_t[:])
                    for g in range(G):
                        sc = spool.tile([128, CHUNK, S], bf16, tag="sc", name="sc")
                        ins = nc.vector._custom_dve(
                            op,
                            out=sc[:],
                            in0=h_tiles[g][:].broadcast_to([128, CHUNK, S]),
                            in1=nrep[:],
                        )
                        ins.ins.perf_max = 1 if USE_2X else 0
                        nc.scalar.copy(
                            out_tiles[g][:, c * CHUNK : (c + 1) * CHUNK],
                            sc[:, :, S - 1 : S],
                        )

            if repeat == 1:
                body()
            else:
                with tc.For_i(0, repeat):
                    body()

            for g in range(G):
                nc.sync.dma_start(
                    outT.ap()[g * 128 : (g + 1) * 128, :], out_tiles[g][:]
                )

    nc.compile()
    return nc


def _get_nc():
    global _NC
    if _NC is None:
        _NC = _build_nc()
    return _NC


def _make_in_maps(h, mention_masks):
    h = np.asarray(h, dtype=np.float32)
    masks = np.asarray(mention_masks)
    neg = np.where(masks == 0, np.float32(-1e30), np.float32(0.0)).astype(np.float32)
    hT = np.ascontiguousarray(h.transpose(0, 2, 1)).astype(ml_dtypes.bfloat16)
    in_maps = []
    for core in range(N_CORES):
        b, mc = divmod(core, N_CORES // B)
        in_maps.append(
            {
                "ht": hT[b],
                "neg": np.ascontiguousarray(
                    neg[b, mc * M_PER_CORE : (mc + 1) * M_PER_CORE]
                )
                .reshape(1, -1)
                .astype(ml_dtypes.bfloat16),
            }
        )
    return in_maps


def kernel(h, mention_masks, trace=False):
    global _LAST_RESULTS
    from concourse.bass_utils import run_bass_kernel_spmd

    nc = _get_nc()
    in_maps = _make_in_maps(h, mention_masks)
    res = run_bass_kernel_spmd(
        nc, in_maps, core_ids=list(range(N_CORES)), trace=trace
    )
    _LAST_RESULTS = res
    out = np.empty((B, M, H), dtype=np.float32)
    for core in range(N_CORES):
        b, mc = divmod(core, N_CORES // B)
        out[b, mc * M_PER_CORE : (mc + 1) * M_PER_CORE] = res.results[core]["outt"].T
    return out
